# revision 26
# baseline (speedup 1.0000x reference)
"""Trainium2 Bass kernel for nn_DiffusionClassifier (dense_mlp).

Data-parallel over batch across 8 NeuronCores (128 samples/core, params
replicated).  Per core:
  conv backbone (3x conv3x3+BN+ReLU+maxpool) as shifted matmuls,
  forward diffusion x_t, per-class 3-layer MLP.

Key optimizations vs v1:
  - error term computed algebraically:  ||n - s||^2 = ||n||^2 - 2 n.s + ||s||^2
    with  n.s = h2 . (W3 n^T)  and  ||s||^2 = h2 G h2^T,  G = W3 W3^T (host).
    This removes the [128,4096]x10 subtract/square vector work entirely.
  - pooling via pairwise tensor_max (2x fewer DVE cycles than reduce_max),
    split across DVE (PSUM evac) / GPSIMD (2nd stage) / ACT (bias+relu).
  - per-(g,k)-bank conv accumulation (short PSUM bank lifetimes), K-split
    row-tiling on conv3/L1/v so LDWEIGHTS hides under the opposite tile.
  - class-paired N=512 matmuls in L1/v; fp8 weights (scale-folded) halve
    weight DMA; all weight streams overlap compute.

Self-contained: hardcodes shapes; host-side numpy does only O(B*F) prep.
"""

import sys
import os

sys.path.insert(0, "/opt/trn_rl_repo")

import numpy as np
import ml_dtypes

import concourse.bass as bass
import concourse.tile as tile
import concourse.mybir as mybir
from concourse import bacc
from concourse.masks import make_identity
from concourse.bass_utils import run_bass_kernel_spmd

F32 = mybir.dt.float32
AL = mybir.AluOpType
AF = mybir.ActivationFunctionType
if os.environ.get("K16", "f16") == "f16":
    F16 = mybir.dt.float16
    F16_NP = np.float16
else:
    F16 = mybir.dt.bfloat16
    F16_NP = ml_dtypes.bfloat16

# dtype knobs (env-overridable for debugging): fp8 weights / fp8 X2
if os.environ.get("KWDT", "f8") == "f8":
    F8 = mybir.dt.float8e4
    F8_NP = ml_dtypes.float8_e4m3
else:
    F8 = mybir.dt.float16
    F8_NP = np.float16
KSTAGE = int(os.environ.get("KSTAGE", "5"))
if os.environ.get("KX2DT", "f8") == "f8":
    X2DT = mybir.dt.float8e4
else:
    X2DT = mybir.dt.float16

NCORES = 8
BC = 128  # samples per core
NCLS, HID, FEAT, TDIM = 10, 256, 4096, 128
BN_EPS = 1e-5
W1S = 64.0  # fp8 scale for sW1
W3S = 16.0  # fp8 scale for sW3

# device sample order: sdev = 64*g + slot ; global-in-core b = 8*(slot//4) + 4*g + slot%4
_PERM = np.array(
    [8 * (s // 4) + 4 * g + (s % 4) for g in (0, 1) for s in range(64)], dtype=np.int64
)

_BUILD_CACHE = {}


def _build(has_b2: bool, has_b3: bool):
    nc = bacc.Bacc("TRN2", target_bir_lowering=False, debug=False, enable_asserts=True)

    d = {}
    def din(name, shape, dt):
        d[name] = nc.dram_tensor(name, list(shape), dt, kind="ExternalInput").ap()

    din("xim", (16, 2, 27, 4, 32, 32), F16)
    din("npre", (128, 32, 8, 16), F16)
    din("noiseT", (128, 32, 128), F16)
    din("timeT", (128, 128), F16)
    din("sa_full", (128, 128), F32)
    din("negsc", (128, 1), F32)
    din("nnsc", (128, NCLS), F32)
    din("w1t", (128, 64), F16)
    din("b1e", (128, 1), F32)
    din("w2t", (9, 128, 128), F16)
    din("b2e", (128, 1), F32)
    din("w3t", (9, 128, 256), F16)
    din("b3e", (128, 2), F32)
    din("sW1q", (5, 33, 128, 512), F8)
    din("sb1d", (128, 2 * NCLS), F32)
    din("sW2t", (NCLS, 2, 128, 256), F16)
    din("sb2d", (128, 2 * NCLS), F32)
    din("sW3q", (5, 32, 128, 512), F8)
    din("Gq", (NCLS, 2, 128, 256), F16)
    if has_b2:
        din("sb2row", (1, NCLS, 256), F16)
    if has_b3:
        din("wbrow", (1, NCLS, 256), F16)
    out_d = nc.dram_tensor("out", [128, NCLS], F32, kind="ExternalOutput").ap()

    with tile.TileContext(nc) as tc:
        with tc.tile_pool(name="consts", bufs=1) as consts, tc.tile_pool(
            name="arena", bufs=1
        ) as arena:
            # ---- constants in SBUF ----
            w1t_s = consts.tile([128, 64], F16)
            nc.sync.dma_start(w1t_s[:], d["w1t"])
            b1e_s = consts.tile([128, 1], F32)
            nc.sync.dma_start(b1e_s[:], d["b1e"])
            w2t_s = consts.tile([128, 9, 128], F16)
            nc.sync.dma_start(w2t_s[:], d["w2t"].rearrange("s c o -> c s o"))
            b2e_s = consts.tile([128, 1], F32)
            nc.sync.dma_start(b2e_s[:], d["b2e"])
            w3t_s = consts.tile([128, 9, 256], F16)
            nc.sync.dma_start(w3t_s[:], d["w3t"].rearrange("s c o -> c s o"))
            b3e_s = consts.tile([128, 2], F32)
            nc.sync.dma_start(b3e_s[:], d["b3e"])
            # big/late-phase consts are allocated here but their DMAs are
            # issued phase-locally below so the first xim tiles land first.
            sb1d_s = consts.tile([128, 2 * NCLS], F32)
            sb2d_s = consts.tile([128, 2 * NCLS], F32)
            sa_s = consts.tile([128, 128], F32)
            negsc_s = consts.tile([128, 1], F32)
            nnsc_s = consts.tile([128, NCLS], F32)
            sW2_s = consts.tile([128, NCLS, 2, 256], F16)
            Gq_s = consts.tile([128, NCLS, 2, 256], F16)
            noiseT_s = consts.tile([128, 32, 128], F16)
            npre_s = consts.tile([128, 32, 8, 16], F16)
            if has_b2:
                sb2r_s = consts.tile([1, NCLS, 256], F16)
                nc.sync.dma_start(sb2r_s[:], d["sb2row"])
            if has_b3:
                wbr_s = consts.tile([1, NCLS, 256], F16)
                nc.sync.dma_start(wbr_s[:], d["wbrow"])
            if has_b2 or has_b3:
                ones1 = consts.tile([1, 128], F16)
                nc.gpsimd.memset(ones1[:], 1.0)
            ident = consts.tile([128, 128], F32)

            # persistent activations
            X2 = arena.tile([128, 64, 18, 18], X2DT, name="X2")
            X3 = arena.tile([128, 128, 10, 10], F16, name="X3")
            # zero only the halo borders (interior fully overwritten)
            nc.gpsimd.memset(X2[:, :, 0, :], 0.0)
            nc.gpsimd.memset(X2[:, :, 17, :], 0.0)
            nc.gpsimd.memset(X2[:, :, 1:17, 0], 0.0)
            nc.gpsimd.memset(X2[:, :, 1:17, 17], 0.0)
            nc.gpsimd.memset(X3[:, :, 0, :], 0.0)
            nc.gpsimd.memset(X3[:, :, 9, :], 0.0)
            nc.gpsimd.memset(X3[:, :, 1:9, 0], 0.0)
            nc.gpsimd.memset(X3[:, :, 1:9, 9], 0.0)

            combT_bf = arena.tile([128, 33, 128], F16, name="combT_bf")
            h1T_all = arena.tile([128, NCLS, 2, 128], F16, name="h1T_all")
            h2_all = arena.tile([128, NCLS, 256], F16, name="h2_all")
            h2T_all = arena.tile([128, NCLS, 2, 128], F16, name="h2T_all")
            qcol = arena.tile([128, NCLS], F32, name="qcol")
            logits = arena.tile([128, NCLS], F32, name="logits")
            nc.vector.memset(logits[:], 0.0)

            # ===== stages 1+2 interleaved per bc: conv 3->64 then conv 64->128 ==
            # s1(bc) fills X2 slots 4bc..4bc+3; s2 k=2bc,2bc+1 consumes them.
            # PE (s2 matmuls) overlaps DVE (s1 PSUM evac).
            with tc.tile_pool(name="io27", bufs=3) as io27, tc.tile_pool(
                name="ev1", bufs=6
            ) as ev1, tc.tile_pool(name="ev2", bufs=4) as ev2, tc.tile_pool(
                name="ps1", bufs=4, space="PSUM"
            ) as ps1, tc.tile_pool(name="ps2", bufs=4, space="PSUM") as ps2:
                for bc in range(16):
                    X27 = io27.tile([128, 4, 32, 32], F16, tag="X27")
                    for g in (0, 1):
                        nc.sync.dma_start(
                            X27[32 * g : 32 * g + 27, :, :, :],
                            d["xim"][bc, g],
                        )
                    # s2 works on the PREVIOUS bc's X2 slots, one shift per
                    # s1 iteration, so the PE never waits on s1 PSUM evac.
                    ks = (2 * bc - 2, 2 * bc - 1) if bc > 0 else ()
                    pg = {}
                    for k in ks:
                        for g in (0, 1):
                            pg[(k, g)] = ps2.tile(
                                [128, 2, 16, 16], F32, tag="s2ps", name=f"s2ps_{g}"
                            )

                    def s2_shift(s, ks=ks, pg=pg):
                        di, dj = divmod(s, 3)
                        for k in ks:
                            for g in (0, 1):
                                nc.tensor.matmul(
                                    pg[(k, g)][:],
                                    w2t_s[64 * g : 64 * g + 64, s, :],
                                    X2[
                                        64 * g : 64 * g + 64,
                                        2 * k : 2 * k + 2,
                                        di : di + 16,
                                        dj : dj + 16,
                                    ],
                                    start=(s == 0),
                                    stop=(s == 8),
                                    skip_group_check=True,
                                )

                    for n in range(8):
                        bq, ih = n // 2, n % 2
                        ps = ps1.tile([128, 16, 32], F32, tag="s1ps")
                        nc.tensor.matmul(
                            ps[0:64],
                            w1t_s[0:27, :],
                            X27[0:27, bq, 16 * ih : 16 * ih + 16, :],
                            start=True,
                            stop=True,
                        )
                        nc.tensor.matmul(
                            ps[64:128],
                            w1t_s[32:59, :],
                            X27[32:59, bq, 16 * ih : 16 * ih + 16, :],
                            tile_position=(32, 64),
                            start=True,
                            stop=True,
                        )
                        if ks:
                            s2_shift(n)
                        slot = 4 * bc + bq
                        # 2x2 maxpool in one DVE reduce (single PSUM read)
                        pv = ps[:].rearrange("p (i a) (j b) -> p i j a b", a=2, b=2)
                        tB = ev1.tile([128, 8, 16], F32, tag="s1b")
                        nc.vector.reduce_max(tB[:], pv, axis=mybir.AxisListType.XY)
                        nc.scalar.activation(
                            X2[:, slot, 1 + 8 * ih : 9 + 8 * ih, 1:17],
                            tB[:],
                            AF.Relu,
                            bias=b1e_s[:, 0:1],
                        )
                    if ks:
                        s2_shift(8)

                    def s2_evac(ks, pg):
                        for k in ks:
                            for g in (0, 1):
                                tB = ev2.tile([128, 2, 8, 8], F32, tag="s2b")
                                for si in (0, 1):
                                    pv = pg[(k, g)][:, si, :, :].rearrange(
                                        "p (i a) (j b) -> p i j a b", a=2, b=2
                                    )
                                    nc.vector.reduce_max(
                                        tB[:, si, :, :],
                                        pv,
                                        axis=mybir.AxisListType.XY,
                                    )
                                sd = 64 * g + 2 * k
                                nc.scalar.activation(
                                    X3[:, sd : sd + 2, 1:9, 1:9],
                                    tB[:],
                                    AF.Relu,
                                    bias=b2e_s[:, 0:1],
                                )

                    if ks:
                        s2_evac(ks, pg)
                # tail: s2 for the last bc's slots (k = 30, 31)
                ks = (30, 31)
                pg = {}
                for k in ks:
                    for g in (0, 1):
                        pg[(k, g)] = ps2.tile(
                            [128, 2, 16, 16], F32, tag="s2ps", name=f"s2ps_{g}"
                        )
                for s in range(9):
                    di, dj = divmod(s, 3)
                    for k in ks:
                        for g in (0, 1):
                            nc.tensor.matmul(
                                pg[(k, g)][:],
                                w2t_s[64 * g : 64 * g + 64, s, :],
                                X2[
                                    64 * g : 64 * g + 64,
                                    2 * k : 2 * k + 2,
                                    di : di + 16,
                                    dj : dj + 16,
                                ],
                                start=(s == 0),
                                stop=(s == 8),
                                skip_group_check=True,
                            )
                s2_evac(ks, pg)

            # ============ stage 3: conv 128->256 (K-split), pool, x_t -> combT ==
            if KSTAGE >= 2:
             nc.sync.dma_start(npre_s[:], d["npre"])
             nc.sync.dma_start(sa_s[:], d["sa_full"])
             nc.sync.dma_start(combT_bf[:, 32, :], d["timeT"])
             with tc.tile_pool(name="ev3", bufs=4) as ev3, tc.tile_pool(
                name="gv3", bufs=4
            ) as gv3, tc.tile_pool(name="ps3", bufs=6, space="PSUM") as ps3:
                for oh in (0, 1):
                    for k in range(16):
                        psA = ps3.tile([128, 8, 8, 8], F32, tag="s3ps", name="s3psA")
                        psB = ps3.tile([128, 8, 8, 8], F32, tag="s3ps", name="s3psB")
                        for s in range(9):
                            di, dj = divmod(s, 3)
                            lw = w3t_s[:, s, 128 * oh : 128 * oh + 128]
                            nc.tensor.matmul(
                                psA[:],
                                lw[0:64, :],
                                X3[0:64, 8 * k : 8 * k + 8, di : di + 8, dj : dj + 8],
                                start=(s == 0),
                                stop=(s == 8),
                            )
                            nc.tensor.matmul(
                                psB[:],
                                lw[64:128, :],
                                X3[64:128, 8 * k : 8 * k + 8, di : di + 8, dj : dj + 8],
                                start=(s == 0),
                                stop=(s == 8),
                            )
                        tC = gv3.tile([128, 8, 8, 8], F32, tag="s3c")
                        nc.scalar.copy(tC[:], psB[:])
                        tS = ev3.tile([128, 8, 8, 8], F32, tag="s3s")
                        nc.vector.tensor_add(tS[:], psA[:], tC[:])
                        # 2x2 maxpool in one reduce, then ReLU+bias on ACT
                        svv = tS[:].rearrange(
                            "p b (i a) (j c) -> p b i j a c", a=2, c=2
                        )
                        tB = gv3.tile([128, 8, 4, 4], F32, tag="s3b")
                        nc.vector.reduce_max(tB[:], svv, axis=mybir.AxisListType.XY)
                        tR = ev3.tile([128, 8, 4, 4], F32, tag="s3r")
                        nc.scalar.activation(
                            tR[:], tB[:], AF.Relu, bias=b3e_s[:, oh : oh + 1]
                        )
                        # x_t = sa*feat + s1*noise  (per-chunk)
                        sab = (
                            sa_s[:, 8 * k : 8 * k + 8]
                            .unsqueeze(2)
                            .unsqueeze(3)
                            .to_broadcast((128, 8, 4, 4))
                        )
                        nc.vector.tensor_mul(tR[:], tR[:], sab)
                        dest = combT_bf[
                            :, 16 * oh : 16 * oh + 16, 8 * k : 8 * k + 8
                        ].rearrange("p (i j) b -> p b i j", i=4, j=4)
                        npv = npre_s[:, 16 * oh + k, :, :].rearrange(
                            "p b (i j) -> p b i j", i=4, j=4
                        )
                        nc.vector.tensor_add(dest, tR[:], npv)

            # ================= L1 (pair-outer, K-split, fp8 weights) ============
            if KSTAGE >= 3:
             make_identity(nc, ident[:])
             nc.sync.dma_start(sb1d_s[:], d["sb1d"])
             nc.sync.dma_start(sW2_s[:], d["sW2t"].rearrange("c h p n -> p c h n"))
             nc.sync.dma_start(sb2d_s[:], d["sb2d"])
             nc.sync.dma_start(noiseT_s[:], d["noiseT"])
             with tc.tile_pool(name="w1pool", bufs=3) as w1pool, tc.tile_pool(
                name="z1pool", bufs=2
            ) as z1pool, tc.tile_pool(name="psL1", bufs=4, space="PSUM") as psL1, tc.tile_pool(
                name="tpps", bufs=2, space="PSUM"
            ) as tpps:
                for pr in range(5):
                    wts = []
                    for h, (f0, f1) in enumerate(((0, 16), (16, 33))):
                        wt = w1pool.tile([128, f1 - f0, 512], F8, tag="w1s", name=f"w1s_{h}")
                        nc.sync.dma_start(
                            wt[:], d["sW1q"][pr, f0:f1].rearrange("f p n -> p f n")
                        )
                        wts.append((f0, wt))
                    psA = psL1.tile([128, 512], F32, tag="l1ps", name="l1psA")
                    psB = psL1.tile([128, 512], F32, tag="l1ps", name="l1psB")
                    fc_order = [32] + list(range(32))
                    for fci, fc in enumerate(fc_order):
                        f0, wt = wts[0] if fc < 16 else wts[1]
                        nc.tensor.matmul(
                            psA[:],
                            combT_bf[0:64, fc, :],
                            wt[0:64, fc - f0, :],
                            start=(fci == 0),
                            stop=(fci == 32),
                            skip_group_check=True,
                        )
                        nc.tensor.matmul(
                            psB[:],
                            combT_bf[64:128, fc, :],
                            wt[64:128, fc - f0, :],
                            start=(fci == 0),
                            stop=(fci == 32),
                            skip_group_check=True,
                        )
                    zB = z1pool.tile([128, 512], F32, tag="zB")
                    nc.scalar.copy(zB[:], psB[:])
                    z1s = z1pool.tile([128, 512], F32, tag="z1s")
                    nc.vector.tensor_add(z1s[:], psA[:], zB[:])
                    for q in range(4):
                        tp = tpps.tile([128, 128], F32, tag="tp")
                        nc.tensor.transpose(tp[:], z1s[:, 128 * q : 128 * q + 128], ident[:])
                        c, hc = 2 * pr + q // 2, q % 2
                        nc.scalar.activation(
                            h1T_all[:, c, hc, :],
                            tp[:],
                            AF.Relu,
                            bias=sb1d_s[:, 2 * c + hc : 2 * c + hc + 1],
                            scale=1.0 / W1S,
                        )

            # ================= L2 (per class): h2T and h2 =======================
            if KSTAGE >= 4:
             nc.sync.dma_start(Gq_s[:], d["Gq"].rearrange("c k p n -> p c k n"))
             nc.sync.dma_start(negsc_s[:], d["negsc"])
             nc.sync.dma_start(nnsc_s[:], d["nnsc"])
             with tc.tile_pool(name="psL2", bufs=3, space="PSUM") as psL2:
                for c in range(NCLS):
                    z2T = psL2.tile([128, 2, 128], F32, tag="z2T", name="z2T")
                    for kh in (0, 1):
                        for hc in (0, 1):
                            nc.tensor.matmul(
                                z2T[:, kh, :],
                                sW2_s[:, c, hc, 128 * kh : 128 * kh + 128],
                                h1T_all[:, c, hc, :],
                                start=(hc == 0 and kh == 0),
                                stop=(hc == 1 and kh == 1),
                                skip_group_check=True,
                            )
                    for kh in (0, 1):
                        nc.scalar.activation(
                            h2T_all[:, c, kh, :],
                            z2T[:, kh, :],
                            AF.Relu,
                            bias=sb2d_s[:, 2 * c + kh : 2 * c + kh + 1],
                        )
                    z2p = psL2.tile([128, 256], F32, tag="z2p", name="z2p")
                    for hc in (0, 1):
                        nc.tensor.matmul(
                            z2p[:],
                            h1T_all[:, c, hc, :],
                            sW2_s[:, c, hc, :],
                            start=(hc == 0),
                            stop=False if has_b2 else (hc == 1),
                            skip_group_check=True,
                        )
                    if has_b2:
                        nc.tensor.matmul(
                            z2p[:],
                            ones1[:, :],
                            sb2r_s[:, c, :],
                            start=False,
                            stop=True,
                            skip_group_check=True,
                        )
                    nc.scalar.activation(h2_all[:, c, :], z2p[:], AF.Relu)

            # ===== v+G (pair-outer):  y = n@(S*W3^T) + h2@(-S/2*G) [+ b3 row] ===
            if KSTAGE >= 5:
             with tc.tile_pool(name="w3pool", bufs=3) as w3pool, tc.tile_pool(
                name="yCp", bufs=2
            ) as yCp, tc.tile_pool(name="tts", bufs=2) as tts, tc.tile_pool(
                name="psV", bufs=4, space="PSUM"
            ) as psV:
                for pr in range(5):
                    wts = []
                    for h in (0, 1):
                        wt = w3pool.tile([128, 16, 512], F8, tag="w3s", name=f"w3s_{h}")
                        nc.sync.dma_start(
                            wt[:],
                            d["sW3q"][pr, 16 * h : 16 * h + 16].rearrange(
                                "f p n -> p f n"
                            ),
                        )
                        wts.append(wt)
                    vA = psV.tile([128, 512], F32, tag="vps", name="vpsA")
                    vB = psV.tile([128, 512], F32, tag="vps", name="vpsB")
                    for j in range(32):
                        wt = wts[j // 16]
                        nc.tensor.matmul(
                            vA[:],
                            noiseT_s[0:64, j, :],
                            wt[0:64, j % 16, :],
                            start=(j == 0),
                            stop=False,
                            skip_group_check=True,
                        )
                        nc.tensor.matmul(
                            vB[:],
                            noiseT_s[64:128, j, :],
                            wt[64:128, j % 16, :],
                            start=(j == 0),
                            stop=(j == 31),
                            skip_group_check=True,
                        )
                    # accumulate ||s||^2 Gram part (and optional b3 row) into vA
                    for i in (0, 1):
                        c = 2 * pr + i
                        for kc in (0, 1):
                            nc.tensor.matmul(
                                vA[:, 256 * i : 256 * i + 256],
                                h2T_all[:, c, kc, :],
                                Gq_s[:, c, kc, :],
                                start=False,
                                stop=(kc == 1) and not has_b3,
                                skip_group_check=True,
                            )
                        if has_b3:
                            nc.tensor.matmul(
                                vA[:, 256 * i : 256 * i + 256],
                                ones1[:, :],
                                wbr_s[:, c, :],
                                start=False,
                                stop=True,
                                skip_group_check=True,
                            )
                    yB = yCp.tile([128, 512], F32, tag="yB")
                    nc.scalar.copy(yB[:], vB[:])
                    yC = yCp.tile([128, 512], F32, tag="yC")
                    nc.vector.tensor_add(yC[:], vA[:], yB[:])
                    for i in (0, 1):
                        c = 2 * pr + i
                        if os.environ.get("KTTR", "off") == "on":
                            sc = tts.tile([128, 256], F16, tag="ttsc")
                            nc.vector.tensor_tensor_reduce(
                                sc[:],
                                yC[:, 256 * i : 256 * i + 256],
                                h2_all[:, c, :],
                                -2.0 / W3S,
                                0.0,
                                AL.mult,
                                AL.add,
                                accum_out=qcol[:, c : c + 1],
                            )
                        else:
                            sc = tts.tile([128, 256], F32, tag="ttsc")
                            nc.vector.tensor_mul(
                                sc[:], yC[:, 256 * i : 256 * i + 256], h2_all[:, c, :]
                            )
                            nc.scalar.activation(
                                sc[:],
                                sc[:],
                                AF.Copy,
                                scale=-2.0 / W3S,
                                accum_out=qcol[:, c : c + 1],
                            )

                # logits = negsc * (nn + q)  ->  (q * negsc) + nnsc
                if os.environ.get("KSTT", "off") == "on":
                    nc.vector.scalar_tensor_tensor(
                        logits[:], qcol[:], negsc_s[:, 0:1], nnsc_s[:], AL.mult, AL.add
                    )
                else:
                    nc.vector.tensor_scalar_mul(qcol[:], qcol[:], negsc_s[:, 0:1])
                    nc.vector.tensor_add(logits[:], qcol[:], nnsc_s[:])
            nc.sync.dma_start(out_d, logits[:])

    nc.compile()
    return nc


def _host_prep(inputs):
    x = np.asarray(inputs["x"], np.float32)
    noise = np.asarray(inputs["noise"], np.float32)
    t = np.asarray(inputs["t"])
    B = x.shape[0]

    betas = np.linspace(0.0001, 0.02, 10, dtype=np.float32)
    ac = np.cumprod((1.0 - betas).astype(np.float32)).astype(np.float32)
    a_t = ac[t]
    sa = np.sqrt(a_t).astype(np.float32)
    s1 = np.sqrt(1.0 - a_t).astype(np.float32)
    negsc = -((s1 / sa) ** 2)
    nn = (noise.astype(np.float32) ** 2).sum(1)

    half = TDIM // 2
    freqs = np.exp(
        np.arange(half, dtype=np.float32) * (-np.log(10000.0) / (half - 1))
    ).astype(np.float32)
    ang = t.astype(np.float32)[:, None] * freqs[None, :]
    t_emb = np.concatenate([np.sin(ang), np.cos(ang)], axis=1).astype(np.float32)

    xpad = np.zeros((B, 3, 34, 34), np.float32)
    xpad[:, :, 1:33, 1:33] = x
    win = np.lib.stride_tricks.sliding_window_view(xpad, (32, 32), axis=(2, 3))
    xim_all = np.ascontiguousarray(
        win.transpose(0, 2, 3, 1, 4, 5).reshape(B, 27, 32, 32)
    )

    def bnfold(i):
        g, be, m, v, b = (
            np.asarray(inputs[f"g{i}"], np.float32),
            np.asarray(inputs[f"be{i}"], np.float32),
            np.asarray(inputs[f"m{i}"], np.float32),
            np.asarray(inputs[f"v{i}"], np.float32),
            np.asarray(inputs[f"b{i}"], np.float32),
        )
        sc = g / np.sqrt(v + BN_EPS)
        return sc, ((b - m) * sc + be).astype(np.float32)

    sc1, bf1 = bnfold(1)
    sc2, bf2 = bnfold(2)
    sc3, bf3 = bnfold(3)
    w1 = np.asarray(inputs["w1"], np.float32) * sc1[:, None, None, None]
    w2 = np.asarray(inputs["w2"], np.float32) * sc2[:, None, None, None]
    w3 = np.asarray(inputs["w3"], np.float32) * sc3[:, None, None, None]

    w1t = np.zeros((128, 64), np.float32)
    for q in (0, 1):
        for s in range(9):
            di, dj = divmod(s, 3)
            for cch in range(3):
                w1t[32 * q + 3 * s + cch, :] = w1[:, cch, di, dj]
    b1e = np.concatenate([bf1, bf1]).astype(np.float32)[:, None]

    w2t = np.zeros((9, 128, 128), np.float32)
    for s in range(9):
        di, dj = divmod(s, 3)
        w2t[s, 0:64, :] = w2[:, :, di, dj].T
        w2t[s, 64:128, :] = w2[:, :, di, dj].T
    b2e = bf2.astype(np.float32)[:, None]

    w3t = np.zeros((9, 128, 256), np.float32)
    for s in range(9):
        di, dj = divmod(s, 3)
        w3t[s] = w3[:, :, di, dj].T
    b3e = bf3.reshape(2, 128).T.copy().astype(np.float32)

    # f-permutation for device comb layout: chunk j (0..31), partition p:
    #   f = ((j//16)*128 + p)*16 + (j%16)
    jj, pp_ = np.meshgrid(np.arange(32), np.arange(128), indexing="ij")
    fidx = ((jj // 16) * 128 + pp_) * 16 + (jj % 16)  # [32, 128]
    fflat = fidx.reshape(-1)
    fext = np.concatenate([fflat, np.arange(FEAT, FEAT + TDIM)])  # [4224]

    sW1 = np.asarray(inputs["sW1"], np.float32)  # [10, 4224, 256]
    # [pr, fc, p, 256*i + h] = W1S * sW1[2pr+i, fext[fc*128+p], h]
    sW1p = (W1S * sW1[:, fext, :]).reshape(NCLS, 33, 128, HID)
    sW1q = np.ascontiguousarray(
        sW1p.reshape(5, 2, 33, 128, HID).transpose(0, 2, 3, 1, 4).reshape(5, 33, 128, 512)
    ).astype(F8_NP)

    sW2 = np.asarray(inputs["sW2"], np.float32).reshape(NCLS, 2, 128, 256)

    sW3 = np.asarray(inputs["sW3"], np.float32)  # [10, 256, 4096]
    W3q8 = (W3S * sW3).astype(F8_NP)  # [10, 256, 4096] quantized
    W3qf = W3q8.astype(np.float32) / W3S  # dequantized for G
    # v weights: [pr, j, p, 256*i + k] = W3S * sW3q[2pr+i, k, fflat[j*128+p]]
    vW = W3q8[:, :, fflat]  # [10, 256, 4096] fp8 (f in device order)
    sW3q = np.ascontiguousarray(
        vW.reshape(NCLS, 256, 32, 128)
        .transpose(0, 2, 3, 1)  # [c, j, p, k]
        .reshape(5, 2, 32, 128, 256)
        .transpose(0, 2, 3, 1, 4)
        .reshape(5, 32, 128, 512)
    )
    G = np.matmul(W3qf, W3qf.transpose(0, 2, 1))  # [10, 256, 256]
    Gq = np.ascontiguousarray(
        (-(W3S / 2.0) * G).reshape(NCLS, 2, 128, 256)
    ).astype(F16_NP)

    sb1 = np.asarray(inputs["sb1"], np.float32)
    sb2 = np.asarray(inputs["sb2"], np.float32)
    sb3 = np.asarray(inputs["sb3"], np.float32)
    sb1d = np.zeros((128, 2 * NCLS), np.float32)
    sb2d = np.zeros((128, 2 * NCLS), np.float32)
    for c in range(NCLS):
        for hc in (0, 1):
            sb1d[:, 2 * c + hc] = sb1[c, 128 * hc : 128 * hc + 128]
            sb2d[:, 2 * c + hc] = sb2[c, 128 * hc : 128 * hc + 128]
    has_b2 = bool(np.any(sb2))
    has_b3 = bool(np.any(sb3))

    common = dict(
        w1t=w1t.astype(F16_NP),
        b1e=b1e,
        w2t=w2t.astype(F16_NP),
        b2e=b2e,
        w3t=w3t.astype(F16_NP),
        b3e=b3e,
        sW1q=sW1q,
        sb1d=sb1d,
        sW2t=sW2.astype(F16_NP),
        sb2d=sb2d,
        sW3q=sW3q,
        Gq=Gq,
    )
    if has_b2:
        common["sb2row"] = np.ascontiguousarray(sb2.reshape(1, NCLS, 256)).astype(
            F16_NP
        )
    if has_b3:
        # +2 s.b3 term: q += alpha * (-W3S * W3 @ b3) . h2  with alpha = -2/W3S
        wb = np.einsum("ckf,cf->ck", W3qf, sb3)  # [10, 256]
        common["wbrow"] = np.ascontiguousarray(
            (-W3S * wb).reshape(1, NCLS, 256)
        ).astype(F16_NP)
        nn_corr = -2.0 * (noise @ sb3.T) + (sb3**2).sum(1)[None, :]  # [B, 10]
    else:
        nn_corr = None

    in_maps = []
    for k in range(NCORES):
        b0 = k * BC
        pg = b0 + _PERM
        nfm = noise[pg][:, fflat]  # [sdev, (j,p)] feature-major rows
        npre32 = (s1[pg][:, None] * nfm).reshape(128, 32, 128).transpose(2, 1, 0)
        # [p, j(=16oh+ij), s] -> [p, 16oh+k, b, ij]  (s = 8k+b)
        npre = (
            npre32.reshape(128, 2, 16, 16, 8)
            .transpose(0, 1, 3, 4, 2)
            .reshape(128, 32, 8, 16)
        )
        noiseT = nfm.reshape(128, 32, 128).transpose(2, 1, 0)
        nns = negsc[pg][:, None] * (
            nn[pg][:, None] + (nn_corr[pg] if nn_corr is not None else 0.0)
        )
        m = dict(common)
        m["xim"] = np.ascontiguousarray(
            xim_all[b0 : b0 + BC]
            .reshape(16, 2, 4, 27, 32, 32)
            .transpose(0, 1, 3, 2, 4, 5)
        ).astype(F16_NP)
        m["npre"] = np.ascontiguousarray(npre).astype(F16_NP)
        m["noiseT"] = np.ascontiguousarray(noiseT).astype(F16_NP)
        m["timeT"] = np.ascontiguousarray(t_emb[pg].T).astype(F16_NP)
        m["sa_full"] = np.ascontiguousarray(
            np.tile(sa[pg][None, :], (128, 1)), np.float32
        )
        m["negsc"] = negsc[pg].astype(np.float32)[:, None]
        m["nnsc"] = np.ascontiguousarray(
            np.tile(nns, (1, NCLS // nns.shape[1]))
            if nns.shape[1] == 1
            else nns
        ).astype(np.float32)
        in_maps.append(m)
    return in_maps, has_b2, has_b3


def kernel(**inputs):
    in_maps, has_b2, has_b3 = _host_prep(inputs)
    key = (has_b2, has_b3)
    if key not in _BUILD_CACHE:
        _BUILD_CACHE[key] = _build(has_b2, has_b3)
    nc = _BUILD_CACHE[key]
    res = run_bass_kernel_spmd(nc, in_maps, core_ids=list(range(NCORES)))
    out = np.zeros((NCORES * BC, NCLS), np.float32)
    for k in range(NCORES):
        out[k * BC + _PERM] = res.results[k]["out"]
    return out
